# revision 31
# baseline (speedup 1.0000x reference)
"""Multi-head attention (B=2, S=2048, D=1024, H=16) on 8 Trainium2 cores.

Sharding: core c handles batch c//4 and heads 4*(c%4)..4*(c%4)+4 (a
256-channel slice of the QKV projections and of the Wo contraction).
Each core computes an output-projection partial over its 256 channels;
the host sums the 4 partials per batch and adds bo_eff = bo + bv @ Wo.T
(the value bias folds out of attention because softmax rows sum to 1).

All matmul operands are bf16 (fp32r moving operands stream at half rate
and force self-loading weight loads that cannot overlap the matmul; bf16
runs 1 column/cycle with standalone, overlappable LDWEIGHTS).  PSUM
accumulation stays fp32; median rel err vs the fp32 reference ~1.7e-3.

Pipeline (per core, ~250us):
  lead-in   activations arrive host-transposed as [128, 8*S] so each
            half-tensor DMA moves 2MB of 16KB-contiguous per-partition
            runs (near line rate).  kt first on the SWDGE queue, weights
            on the scalar ring, vt on sync, qt SWDGE behind kt (gated by
            stream-pool rotation so completion order matches use order).
            ~60 dummy matmuls keep the PE clock warm until kt lands.
  proj      three passes k, v, q through the 8 PSUM banks; projections
            evict through DVE with the bias add fused (bf16 out).
  attention (q-quarter 512, head-pair, 2-kb window), software-pipelined:
            MM1 kT[64,128].T @ qT -> psc [128k, 1024] (the two heads sit
            on PE row groups 0/64), exp on ScalarE (scale=1/8 fused,
            scores bounded so no max-subtraction) -> ets bf16, MM2
            accumulates po [65, 512] over all 16 k-blocks.  v is stored
            per k-block as [128, 4*65] with a ones column per head so
            MM2's row 64 is the softmax denominator: fast reciprocal +
            partition broadcast + multiply on eviction into ao (bf16).
            ScalarE exp (~137us) and the PE (~150us incl. LDWEIGHTS
            serialization) are both near-saturated here; psc is triple
            buffered (6 banks) + po single (2 banks) = all 8 banks.
  out-proj  tail: ao.T @ wo in 1-bank jc-chunks (pj 6-deep), evictions
            alternate DVE/ScalarE, bf16 partials stream out over SWDGE.
"""

import sys

if "/opt/trn_rl_repo" not in sys.path:
    sys.path.insert(0, "/opt/trn_rl_repo")

import numpy as np

B, S, D = 2, 2048, 1024
HPC = 4            # heads per core
OC = HPC * 64      # projection output channels per core
NCORES = 8
KB = 16            # k-blocks of 128
VSTRIDE = HPC * 65 # v storage per k-block: 4 heads x (64 v + 1 ones)

_CACHE = {}


def _build():
    import concourse.bacc as bacc
    import concourse.mybir as mybir
    from concourse.tile import TileContext

    F32 = mybir.dt.float32
    BF16 = mybir.dt.bfloat16
    AF = mybir.ActivationFunctionType

    nc = bacc.Bacc(None, target_bir_lowering=False)
    # x-streams arrive pre-rearranged on host as [p, t*S]: row p holds the
    # 8 D-dim rows {t*128+p}, so a half-tensor DMA is 128 x 16KB contiguous
    # runs (per-partition descriptor size is what sets DMA efficiency)
    qt_in = nc.dram_tensor("qt", [128, 8 * S], BF16, kind="ExternalInput")
    kt_in = nc.dram_tensor("kt", [128, 8 * S], BF16, kind="ExternalInput")
    vt_in = nc.dram_tensor("vt", [128, 8 * S], BF16, kind="ExternalInput")
    # weights arrive pre-rearranged on host: [p, t*OC] with row p holding
    # D-dim t*128+p, so every weight DMA is a flat contiguous transfer
    wq_in = nc.dram_tensor("wq", [128, 8 * OC], BF16, kind="ExternalInput")
    wk_in = nc.dram_tensor("wk", [128, 8 * OC], BF16, kind="ExternalInput")
    wv_in = nc.dram_tensor("wv", [128, 8 * OC], BF16, kind="ExternalInput")
    wo_in = nc.dram_tensor("wo", [OC, D], BF16, kind="ExternalInput")
    bq_in = nc.dram_tensor("bq", [128, 2], F32, kind="ExternalInput")
    bk_in = nc.dram_tensor("bk", [128, 2], F32, kind="ExternalInput")
    out = nc.dram_tensor("out", [S, D], BF16, kind="ExternalOutput")

    with TileContext(nc) as tc:
        with tc.tile_pool(name="wpool", bufs=1) as wp, \
             tc.tile_pool(name="data", bufs=1) as dp:
            wq_sb = wp.tile([128, 8 * OC], BF16, name="wq_sb")
            wk_sb = wp.tile([128, 8 * OC], BF16, name="wk_sb")
            wv_sb = wp.tile([128, 8 * OC], BF16, name="wv_sb")
            wo_sb = [wp.tile([128, D], BF16, name=f"wo{p}") for p in range(2)]
            bq_sb = wp.tile([128, 2], F32, name="bq_sb")
            bk_sb = wp.tile([128, 2], F32, name="bk_sb")
            wz = wp.tile([128, 512], BF16, name="warm_z")

            # PE warm-up: dummy matmuls on a zeroed tile while the first
            # input DMAs land, so real matmuls start at the warm clock;
            # the warm tile lives in the persistent pool so its region
            # never creates WAR edges against the stream DMAs
            with tc.tile_pool(name="pwarm", bufs=1, space="PSUM") as pwp:
                nc.vector.memset(wz[:], 0.0)
                pwz = pwp.tile([128, 512], F32, name="warm_p")
                for _ in range(60):
                    nc.tensor.matmul(pwz[:], wz[:, 0:128], wz[:],
                                     start=True, stop=True)

            qT = [dp.tile([128, S], BF16, name=f"qT{p}") for p in range(2)]
            kT = [dp.tile([128, S], BF16, name=f"kT{p}") for p in range(2)]
            v_sb = dp.tile([128, KB * VSTRIDE], BF16, name="v_sb")
            ao = [dp.tile([128, S], BF16, name=f"ao{p}") for p in range(2)]

            # ---- projections: three streamed passes (k, v, q) through one
            # ---- rotating stream pool; kt/qt on sync queue, vt on gpsimd ----
            with tc.tile_pool(name="stream", bufs=3) as sp, \
                 tc.tile_pool(name="pproj", bufs=1, space="PSUM") as pp:
                # k pass: kT[o, s] accumulated over 8 i-tiles
                kpts = [[pp.tile([128, 512], F32, name=f"ppk_{ob}_{sc}",
                                 tag=f"pp{ob}{sc}")
                         for sc in range(4)] for ob in range(2)]
                # kt,qt ride the SWDGE queue, vt the sync ring; each tensor
                # moves as two 2MB transfers (4 t-chunks each) -- large
                # per-partition runs keep the DMA near line rate
                def xload(eng, src_t, nm):
                    halves = []
                    for h in range(2):
                        xt = sp.tile([128, 4 * S], BF16, name=f"{nm}{h}", tag="xstream")
                        eng.dma_start(xt[:], src_t[:, h * 4 * S:(h + 1) * 4 * S])
                        halves.append(xt)
                    return [halves[t // 4][:, (t % 4) * S:(t % 4 + 1) * S]
                            for t in range(8)]
                kts = xload(nc.gpsimd, kt_in, "xk")
                # weights next: their DMA-completion semaphore lanes then
                # never gate the k-stream, and wk lands before k-MMs start
                nc.scalar.dma_start(wk_sb[:], wk_in[:, :])
                nc.scalar.dma_start(wq_sb[:], wq_in[:, :])
                nc.scalar.dma_start(wv_sb[:], wv_in[:, :])
                vts = xload(nc.sync, vt_in, "xv")
                nc.scalar.dma_start(bq_sb[:], bq_in[:, :])
                nc.scalar.dma_start(bk_sb[:], bk_in[:, :])
                for p in range(2):
                    nc.scalar.dma_start(wo_sb[p][:],
                                        wo_in[p * 128:(p + 1) * 128, :])
                qts = xload(nc.gpsimd, qt_in, "xq")
                nc.vector.memset(v_sb[:], 1.0)
                for t in range(8):
                    for ob in range(2):
                        w_ap = wk_sb[:, t * OC + ob * 128: t * OC + ob * 128 + 128]
                        for sc in range(4):
                            nc.tensor.matmul(
                                kpts[ob][sc][:], w_ap,
                                kts[t][:, sc * 512:(sc + 1) * 512],
                                start=(t == 0), stop=(t == 7))

                for ob in range(2):
                    for sc in range(4):
                        nc.vector.tensor_scalar_add(
                            kT[ob][:, sc * 512:(sc + 1) * 512],
                            kpts[ob][sc][:], bk_sb[:, ob:ob + 1])

                # v pass: v[s, o] with two s-blocks packed per psum tile
                vps = [pp.tile([128, 512], F32, name=f"vp_{jj}", tag=f"pp{jj % 2}{jj // 2}")
                       for jj in range(8)]
                for t in range(8):
                    for sbk in range(16):
                        nc.tensor.matmul(
                            vps[sbk // 2][:, (sbk % 2) * 256:(sbk % 2) * 256 + 256],
                            vts[t][:, sbk * 128:(sbk + 1) * 128],
                            wv_sb[:, t * OC:(t + 1) * OC],
                            start=(t == 0 and sbk % 2 == 0), stop=(t == 7))
                for sbk in range(16):
                    dst = v_sb[:, sbk * VSTRIDE:(sbk + 1) * VSTRIDE] \
                        .rearrange("p (h c) -> p h c", c=65)[:, :, 0:64]
                    src2 = vps[sbk // 2][:, (sbk % 2) * 256:(sbk % 2) * 256 + 256] \
                        .rearrange("p (h c) -> p h c", c=64)
                    nc.vector.tensor_copy(dst, src2)

                # q pass
                qpts = [[pp.tile([128, 512], F32, name=f"ppq_{ob}_{sc}",
                                 tag=f"pp{ob}{sc}")
                         for sc in range(4)] for ob in range(2)]
                for t in range(8):
                    for ob in range(2):
                        w_ap = wq_sb[:, t * OC + ob * 128: t * OC + ob * 128 + 128]
                        for sc in range(4):
                            nc.tensor.matmul(
                                qpts[ob][sc][:], w_ap,
                                qts[t][:, sc * 512:(sc + 1) * 512],
                                start=(t == 0), stop=(t == 7))
                for ob in range(2):
                    for sc in range(4):
                        nc.vector.tensor_scalar_add(
                            qT[ob][:, sc * 512:(sc + 1) * 512],
                            qpts[ob][sc][:], bq_sb[:, ob:ob + 1])

            # ---- attention: (q-quarter, head-pair, 2-kb window) ----
            with tc.tile_pool(name="pexp", bufs=4) as ep, \
                 tc.tile_pool(name="rl", bufs=2) as rlp, \
                 tc.tile_pool(name="pss", bufs=3, space="PSUM") as pss, \
                 tc.tile_pool(name="pso", bufs=1, space="PSUM") as pso:
                for qq in range(4):           # q quarter
                    q0 = qq * 512
                    for pt in range(2):       # head pair (heads 2pt, 2pt+1)
                        po = [pso.tile([65, 512], F32, name=f"po{pt}_{qq}_{hh}",
                                       tag=f"po{hh}") for hh in range(2)]
                        prev_ets = None
                        for w in range(9):    # 2-kb windows, SW-pipelined by 1
                            if w < 8:
                                psc = [pss.tile([128, 1024], F32,
                                                name=f"psc{pt}_{qq}_{w}_{hh}", tag="psc")
                                       for hh in range(2)]
                                # MM1: the two heads alternate row groups (b0, b64)
                                for kb2 in range(2):
                                    kb = 2 * w + kb2
                                    for hh in range(2):
                                        bp = hh * 64
                                        nc.tensor.matmul(
                                            psc[hh][:, kb2 * 512:(kb2 + 1) * 512],
                                            kT[pt][bp:bp + 64, kb * 128:(kb + 1) * 128],
                                            qT[pt][bp:bp + 64, q0:q0 + 512],
                                            start=True, stop=True)
                            # MM2 batch for the PREVIOUS window: 4 x c128
                            if prev_ets is not None:
                                pw = w - 1
                                for hh in range(2):
                                    h = 2 * pt + hh
                                    for kb2 in range(2):
                                        kb = 2 * pw + kb2
                                        va = v_sb[:, kb * VSTRIDE + h * 65:
                                                  kb * VSTRIDE + h * 65 + 65]
                                        nc.tensor.matmul(
                                            po[hh][:], va,
                                            prev_ets[hh][:, kb2 * 512:(kb2 + 1) * 512],
                                            start=(pw == 0 and kb2 == 0),
                                            stop=(pw == 7 and kb2 == 1))
                            if w < 8:
                                ets = []
                                for hh in range(2):
                                    et = ep.tile([128, 1024], BF16,
                                                 name=f"et{pt}_{qq}_{w}_{hh}", tag="et")
                                    nc.scalar.activation(et[:], psc[hh][:], AF.Exp,
                                                         scale=0.125)
                                    ets.append(et)
                                prev_ets = ets
                        # normalize by the ones-column row sums; the two
                        # heads' chains are interleaved so DVE/GpSimd overlap
                        rl1s, rcps = [], []
                        for hh in range(2):
                            rl0 = rlp.tile([1, 512], F32, name=f"rl0{pt}{qq}{hh}", tag=f"rl0{hh}")
                            nc.vector.tensor_copy(rl0[:], po[hh][64:65, :])
                            rl1 = rlp.tile([1, 512], F32, name=f"rl1{pt}{qq}{hh}", tag=f"rl1{hh}")
                            nc.vector.reciprocal_approx_fast(rl1[:], rl0[:])
                            rl1s.append(rl1)
                        for hh in range(2):
                            rcp = rlp.tile([64, 512], F32, name=f"rcp{pt}{qq}{hh}", tag=f"rcp{hh}")
                            nc.gpsimd.partition_broadcast(rcp[:], rl1s[hh][:])
                            rcps.append(rcp)
                        nc.vector.tensor_mul(ao[pt][0:64, q0:q0 + 512],
                                             po[0][0:64, :], rcps[0][:])
                        tmp = rlp.tile([64, 512], BF16, name=f"tm{pt}{qq}", tag="tm")
                        nc.vector.tensor_mul(tmp[:], po[1][0:64, :], rcps[1][:])
                        nc.gpsimd.dma_start(ao[pt][64:128, q0:q0 + 512], tmp[:])

            # ---- output projection partial: out[s, :] = sum_h ao_h.T @ wo_h
            # ---- jc-chunks keep pj to one PSUM bank; evictions alternate
            # ---- DVE/ScalarE; bf16 out rides the gpsimd SWDGE queue ----
            with tc.tile_pool(name="pj", bufs=6, space="PSUM") as pjp, \
                 tc.tile_pool(name="ostage", bufs=4) as osp:
                for sb in range(16):
                    ot = osp.tile([128, 1024], BF16, name=f"ot{sb}", tag="ot")
                    for jc in range(2):
                        pj = pjp.tile([128, 512], F32, name=f"pj{sb}_{jc}", tag="pj")
                        for pt in range(2):   # full c=128 over the head pair
                            nc.tensor.matmul(
                                pj[:], ao[pt][:, sb * 128:(sb + 1) * 128],
                                wo_sb[pt][:, jc * 512:(jc + 1) * 512],
                                start=(pt == 0), stop=(pt == 1))
                        if jc == 0:
                            nc.vector.tensor_copy(ot[:, 0:512], pj[:])
                        else:
                            nc.scalar.activation(ot[:, 512:1024], pj[:], AF.Copy)
                    nc.gpsimd.dma_start(out[sb * 128:(sb + 1) * 128, :], ot[:])

    nc.finalize()
    return nc


def _get_nc():
    if "nc" not in _CACHE:
        _CACHE["nc"] = _build()
    return _CACHE["nc"]


def _bf16(x):
    import ml_dtypes
    return np.ascontiguousarray(x).astype(ml_dtypes.bfloat16)


def _wlayout(WT):
    # [D, OC] -> [128, 8*OC] with row p holding D-dims {t*128+p: t=0..7}
    return _bf16(WT.reshape(8, 128, OC).transpose(1, 0, 2).reshape(128, 8 * OC))


def _xlayout(X):
    # [S, D] activations -> [128, 8*S]: row p holds D-dims {t*128+p: t=0..7}
    return _bf16(X.T.reshape(8, 128, S).transpose(1, 0, 2).reshape(128, 8 * S))


def _in_maps(Q, K, V, Wq, bq, Wk, bk, Wv, bv, Wo, bo):
    maps = []
    qkvt = [(_xlayout(Q[b]), _xlayout(K[b]), _xlayout(V[b])) for b in range(B)]
    for c in range(NCORES):
        b, g = c // 4, c % 4
        sl = slice(g * OC, (g + 1) * OC)
        maps.append({
            "qt": qkvt[b][0],
            "kt": qkvt[b][1],
            "vt": qkvt[b][2],
            "wq": _wlayout(Wq[sl, :].T),
            "wk": _wlayout(Wk[sl, :].T),
            "wv": _wlayout(Wv[sl, :].T),
            "wo": _bf16(Wo[:, sl].T),
            "bq": np.ascontiguousarray(bq[sl].reshape(2, 128).T.astype(np.float32)),
            "bk": np.ascontiguousarray(bk[sl].reshape(2, 128).T.astype(np.float32)),
        })
    return maps


def kernel(Q, K, V, Wq, bq, Wk, bk, Wv, bv, Wo, bo, validate=False, **_kw):
    from concourse.bass_utils import run_bass_kernel_spmd

    Q, K, V = (np.asarray(x, np.float32) for x in (Q, K, V))
    Wq, bq, Wk, bk, Wv, bv, Wo, bo = (
        np.asarray(x, np.float32) for x in (Wq, bq, Wk, bk, Wv, bv, Wo, bo))

    nc = _get_nc()
    res = run_bass_kernel_spmd(nc, _in_maps(Q, K, V, Wq, bq, Wk, bk, Wv, bv, Wo, bo),
                               core_ids=list(range(NCORES)))
    parts = [res.results[c]["out"] for c in range(NCORES)]
    bo_eff = (bo.astype(np.float64) + bv.astype(np.float64) @ Wo.T.astype(np.float64)
              ).astype(np.float32)
    outs = []
    for b in range(B):
        acc = parts[4 * b].astype(np.float32)
        for g in range(1, 4):
            acc += parts[4 * b + g].astype(np.float32)
        outs.append(acc + bo_eff)
    return np.stack(outs)
